# revision 11
# baseline (speedup 1.0000x reference)
"""Bass/Trainium2 kernel for nn_Attention_84688165142614 (additive attention).

Computes, for full inputs (B=32, S=2048, EH=512, DH=512):
    enc    = enc_output.transpose(1, 0, 2)                  # [B, S, 2EH]
    energy = tanh(enc @ w_enc + (h @ w_dec) + attn_b)       # [B, S, DH]
    att    = energy @ v_w                                   # [B, S]
    att    = where(mask == 0, -1e10, att)
    out    = softmax(att, axis=1)

Strategy: data-parallel over batch across 8 NeuronCores (4 batches/core),
plus mask-sparsity compaction. The mask is ~50% zeros and masked positions
produce exactly 0 in the reference output (exp(-1e10) underflows in f32),
so the host keeps only unmasked source positions per batch (gather),
pads each batch to a multiple of 128 columns, transposes the kept enc
columns feature-major and pre-casts to bf16. Batches are assigned to
(core, slot) by sorted compacted width, so the SPMD per-slot tile counts
are the max over cores of the k-th widest batch — for a p=0.5 mask this
drops one padded 128-column tile from most slots. The device computes
energies/logits/softmax only for the compacted columns (pads are killed
with a -1e10 additive mask row), and the host scatters the compacted
probabilities back into a zero [B, S] output.

Device structure: the big matmul runs in bf16 with fp32 PSUM accumulation.
The enc DRAM shard is laid out group-major — exactly the order the PE
consumes it (per slot, per PSUM group of 4 s-tiles, ec-major inside) —
so each (slot, group) is one contiguous DMA into its own SBUF tile, all
on the single sync HWDGE ring in consumption order (SDMA round-robins
across rings at packet granularity, so a single FIFO ring is the only
way to prioritize; it still spreads across all 16 SDMA engines). Warmup
matmuls on memset data keep the PE busy (and the HAM clock un-throttled)
during the initial fill. tanh output and v are bf16. Softmax skips
max-subtraction (logits are bounded by sum|v| ~ 8); all logits are
shifted by -8*ln2 (softmax is shift-invariant) so exp sums fit fp16,
letting the partition-sum broadcast matmul run 1-pass fp16 instead of
2-pass fp32. The last slot folds the dec add into the matmul
accumulation so the vector backlog drains before the kernel tail.
"""

import numpy as np
from contextlib import ExitStack

import concourse.bass as bass
import concourse.tile as tile
from concourse import bacc, mybir
from concourse.bass_utils import run_bass_kernel_spmd

# Problem shape (hardcoded; kernel.py must be self-contained).
B, S, E2, DH = 32, 2048, 1024, 512
N_CORES = 8
BC = B // N_CORES        # batches per core = 4
P = 128                  # SBUF partitions
EC = E2 // P             # enc-feature chunks = 8
D = DH                   # 512
KC = DH // P             # dec-feature chunks = 4

f32 = mybir.dt.float32
bf16 = mybir.dt.bfloat16
fp16 = mybir.dt.float16
AF = mybir.ActivationFunctionType
ALU = mybir.AluOpType

NEG_BIG = -1.0e10
SHIFT = 8.0 * 0.6931471805599453  # logit shift so exp sums fit fp16

_NC_CACHE = {}


def _group_sizes(nt):
    sizes = [4] * (nt // 4)
    if nt % 4:
        sizes.append(nt % 4)
    return sizes


def _emit(ctx, tc, nc, widths, enc_t, hwdec, madd_in, w_enc, sel_in, bv_in, out):
    nslots = len(widths)
    toff = [sum(widths[:i]) for i in range(nslots)]        # tile offset per slot
    ntot = sum(widths)
    slot_sizes = [_group_sizes(w) for w in widths]
    # column offset (in the EC-major enc layout) of (slot, group)
    coff = {}
    c = 0
    for b in range(nslots):
        for sg, gsz in enumerate(slot_sizes[b]):
            coff[(b, sg)] = c
            c += EC * gsz * P

    const = ctx.enter_context(tc.tile_pool(name="const", bufs=1))
    spsum = ctx.enter_context(tc.tile_pool(name="spsum", bufs=1, space="PSUM"))
    mpsum = ctx.enter_context(tc.tile_pool(name="mpsum", bufs=7, space="PSUM"))
    ngrp = sum(len(s) for s in slot_sizes)
    encp = ctx.enter_context(tc.tile_pool(name="encp", bufs=ngrp))
    tmpp = ctx.enter_context(tc.tile_pool(name="tmpp", bufs=3))
    thp = ctx.enter_context(tc.tile_pool(name="thp", bufs=5))
    epip = ctx.enter_context(tc.tile_pool(name="epip", bufs=12))

    # ---- warmup source tiles (no DMA deps): keep the PE busy during fill ----
    ones16 = const.tile([P, P], fp16)
    nc.vector.memset(ones16[:], 1.0)
    wsrc = const.tile([P, D], fp16)
    nc.vector.memset(wsrc[:], 0.001)
    ones_row = const.tile([1, P], bf16)
    nc.vector.memset(ones_row[:], 1.0)

    # ---- DMA: single sync ring in exact consumption order ----
    gtiles = {}
    for b in range(nslots):
        for sg, gsz in enumerate(slot_sizes[b]):
            gtiles[(b, sg)] = encp.tile(
                [P, EC * gsz * P], bf16, tag="enc", name=f"enc_{b}_{sg}"
            )

    wq = const.tile([P, EC * D], bf16)
    hwdec_sb = const.tile([P, KC * BC + KC * D], bf16)
    madd_sb = const.tile([P, ntot], f32)
    sel_sb = const.tile([BC, BC * P], bf16)
    bv_sb = const.tile([1, 2 * D], bf16)

    g00 = gtiles[(0, 0)]
    Wg0 = slot_sizes[0][0] * P
    # fine-grained opening: wq/enc chunk pairs in consumption order
    nc.sync.dma_start(out=wq[:, 0:D], in_=w_enc[:, 0:D])
    nc.sync.dma_start(out=g00[:, 0:Wg0], in_=enc_t[:, 0:Wg0])
    nc.sync.dma_start(out=wq[:, D : 2 * D], in_=w_enc[:, D : 2 * D])
    nc.sync.dma_start(out=g00[:, Wg0 : 2 * Wg0], in_=enc_t[:, Wg0 : 2 * Wg0])
    nc.sync.dma_start(out=wq[:, 2 * D : 4 * D], in_=w_enc[:, 2 * D : 4 * D])
    nc.sync.dma_start(out=g00[:, 2 * Wg0 : 4 * Wg0], in_=enc_t[:, 2 * Wg0 : 4 * Wg0])
    nc.sync.dma_start(out=wq[:, 4 * D : 8 * D], in_=w_enc[:, 4 * D : 8 * D])
    nc.sync.dma_start(out=g00[:, 4 * Wg0 : 8 * Wg0], in_=enc_t[:, 4 * Wg0 : 8 * Wg0])

    # small consts on the scalar ring (kept free of large transfers)
    nc.scalar.dma_start(out=hwdec_sb[:], in_=hwdec[:])
    nc.scalar.dma_start(out=madd_sb[:], in_=madd_in[:])
    nc.scalar.dma_start(out=sel_sb[:], in_=sel_in[:])
    nc.scalar.dma_start(out=bv_sb[:], in_=bv_in[:])

    # remaining groups on the sync ring in consumption order
    rest = [(b, sg) for b in range(nslots) for sg in range(len(slot_sizes[b]))][1:]
    for b, sg in rest:
        gsz = slot_sizes[b][sg]
        lo = coff[(b, sg)]
        hi = lo + EC * gsz * P
        nc.sync.dma_start(out=gtiles[(b, sg)][:], in_=enc_t[:, lo:hi])

    # ---- PE warmup: 8 matmuls on memset data (~3.4us cold, HAM -> 8/8) ----
    wps = spsum.tile([P, D], f32, tag="sp", name="warm")
    for i in range(8):
        nc.tensor.matmul(wps[:], lhsT=ones16[:], rhs=wsrc[:], start=True, stop=True)

    # ---- dec[b, :] = h[slot b] @ w_dec + attn_b; broadcasts ----
    HW0 = KC * BC  # offset of w_dec columns inside hwdec_sb
    dec_ps = spsum.tile([BC, D], f32, tag="sp")
    for kc in range(KC):
        nc.tensor.matmul(
            dec_ps[:],
            lhsT=hwdec_sb[:, kc * BC : (kc + 1) * BC],
            rhs=hwdec_sb[:, HW0 + kc * D : HW0 + (kc + 1) * D],
            start=(kc == 0),
            stop=False,
        )
    nc.tensor.matmul(
        dec_ps[:], lhsT=ones_row[:, 0:BC], rhs=bv_sb[:, 0:D], start=False, stop=True
    )
    dec_rows = const.tile([BC, D], bf16)
    nc.vector.tensor_copy(dec_rows[:], dec_ps[:])

    dec_bc = const.tile([P, BC * D], f32)
    for b in range(nslots):
        ps = spsum.tile([P, D], f32, tag="sp", name=f"decb_{b}")
        nc.tensor.matmul(
            ps[:], lhsT=sel_sb[:, b * P : (b + 1) * P], rhs=dec_rows[:],
            start=True, stop=True,
        )
        nc.vector.tensor_copy(dec_bc[:, b * D : (b + 1) * D], ps[:])
    v_ps = spsum.tile([P, D], f32, tag="sp")
    nc.tensor.matmul(
        v_ps[:], lhsT=ones_row[:], rhs=bv_sb[:, D : 2 * D], start=True, stop=True
    )
    v_sb = const.tile([P, D], bf16)
    nc.vector.tensor_copy(v_sb[:], v_ps[:])

    # ---- main loop over slots ----
    for b in range(nslots):
        nt = widths[b]
        sizes = slot_sizes[b]
        starts = [sum(sizes[:i]) for i in range(len(sizes))]
        # last slot: dec add folded into the accumulation so tanh reads PSUM
        # and the vector backlog drains before the kernel tail
        dec_in_mm = b == nslots - 1
        att = epip.tile([P, nt], f32, tag="att", name=f"att_{b}")
        for sg, gsz in enumerate(sizes):
            gt = gtiles[(b, sg)]
            Wg = gsz * P
            psums = [
                mpsum.tile([P, D], f32, tag="mm", name=f"mm_{b}_{sg}_{j}")
                for j in range(gsz)
            ]
            for ec in range(EC):
                for j in range(gsz):
                    nc.tensor.matmul(
                        psums[j][:],
                        lhsT=gt[:, ec * Wg + j * P : ec * Wg + (j + 1) * P],
                        rhs=wq[:, ec * D : (ec + 1) * D],
                        start=(ec == 0),
                        stop=(ec == EC - 1) and not dec_in_mm,
                    )
            if dec_in_mm:
                for j in range(gsz):
                    nc.tensor.matmul(
                        psums[j][:],
                        lhsT=sel_sb[:, b * P : (b + 1) * P],
                        rhs=dec_rows[:],
                        start=False,
                        stop=True,
                    )
            for j in range(gsz):
                st = starts[sg] + j
                th = thp.tile([P, D], bf16, tag="th")
                if dec_in_mm:
                    nc.scalar.activation(th[:], psums[j][:], AF.Tanh)
                else:
                    t_sb = tmpp.tile([P, D], f32, tag="tmp")
                    nc.vector.tensor_add(
                        t_sb[:], psums[j][:], dec_bc[:, b * D : (b + 1) * D]
                    )
                    nc.scalar.activation(th[:], t_sb[:], AF.Tanh)
                scr = thp.tile([P, D], bf16, tag="scr")
                nc.vector.affine_mul_reduce(
                    out=scr[:],
                    accum_out=att[:, st : st + 1],
                    in0=th[:],
                    in1=v_sb[:],
                    scale=1.0,
                    bias=0.0,
                )

        # ---- epilogue: mask+shift, exp, fp16 partition-sum bcast, scale ----
        attm = epip.tile([P, nt], f32, tag="attm", name=f"attm_{b}")
        nc.vector.tensor_add(
            attm[:], att[:], madd_sb[:, toff[b] : toff[b] + nt]
        )
        expt = epip.tile([P, nt], f32, tag="expt", name=f"expt_{b}")
        partial = epip.tile([P, 1], f32, tag="partial", name=f"psum_{b}")
        nc.scalar.activation(expt[:], attm[:], AF.Exp)
        nc.vector.tensor_reduce(partial[:], expt[:], mybir.AxisListType.X, ALU.add)
        p16 = epip.tile([P, 1], fp16, tag="p16", name=f"p16_{b}")
        nc.vector.tensor_copy(p16[:], partial[:])
        tot_ps = spsum.tile([P, 1], f32, tag="sp", name=f"tot_{b}")
        nc.tensor.matmul(
            tot_ps[:], lhsT=ones16[:], rhs=p16[:], start=True, stop=True
        )
        r_pp = epip.tile([P, 1], f32, tag="rpp", name=f"rpp_{b}")
        nc.vector.reciprocal(r_pp[:], tot_ps[:])
        out_sb = epip.tile([P, nt], f32, tag="outsb", name=f"osb_{b}")
        nc.vector.tensor_scalar_mul(out_sb[:], expt[:], r_pp[:])
        nc.scalar.dma_start(
            out=out[:, toff[b] : toff[b] + nt], in_=out_sb[:]
        )


def build_nc(widths):
    key = tuple(widths)
    if key in _NC_CACHE:
        return _NC_CACHE[key]
    ntot = sum(widths)
    nc = bacc.Bacc("TRN2", target_bir_lowering=False, debug=False)
    enc_t = nc.dram_tensor(
        "enc_t", [P, EC * P * ntot], bf16, kind="ExternalInput"
    ).ap()
    hwdec = nc.dram_tensor(
        "hwdec", [P, KC * BC + KC * D], bf16, kind="ExternalInput"
    ).ap()
    madd = nc.dram_tensor("madd", [P, ntot], f32, kind="ExternalInput").ap()
    w_enc = nc.dram_tensor("w_enc", [P, EC * D], bf16, kind="ExternalInput").ap()
    sel_in = nc.dram_tensor("sel_in", [BC, BC * P], bf16, kind="ExternalInput").ap()
    bv = nc.dram_tensor("bv", [1, 2 * D], bf16, kind="ExternalInput").ap()
    out = nc.dram_tensor("out", [P, ntot], f32, kind="ExternalOutput").ap()

    with tile.TileContext(nc) as tc:
        with ExitStack() as ctx:
            _emit(ctx, tc, nc, list(widths), enc_t, hwdec, madd, w_enc, sel_in,
                  bv, out)
    nc.compile()
    _NC_CACHE[key] = nc
    return nc


def plan_assignment(counts):
    """Sort batches by compacted tile count; rank k -> core k%8, slot k//8.
    Returns (assign[core][slot] = global batch, widths[slot])."""
    tiles = np.maximum(1, np.ceil(counts / P).astype(int))
    order = sorted(range(B), key=lambda gb: (-tiles[gb], -counts[gb], gb))
    assign = [[-1] * BC for _ in range(N_CORES)]
    widths = []
    for slot in range(BC):
        ranks = order[slot * N_CORES : (slot + 1) * N_CORES]
        for c, gb in enumerate(ranks):
            assign[c][slot] = gb
        widths.append(max(int(tiles[gb]) for gb in ranks))
    return assign, widths


def shard_inputs(inputs, assign, widths):
    import ml_dtypes

    h = np.asarray(inputs["h"], dtype=np.float32)
    enc = np.asarray(inputs["enc_output"], dtype=np.float32)
    mask = np.asarray(inputs["mask"], dtype=np.int32)
    attn_w = np.asarray(inputs["attn_w"], dtype=np.float32)
    attn_b = np.asarray(inputs["attn_b"], dtype=np.float32)
    v_w = np.asarray(inputs["v_w"], dtype=np.float32)

    ntot = sum(widths)
    toff = [sum(widths[:i]) for i in range(len(widths))]

    # w_dec [DH, D] -> [P, KC*D] with free index (kc, d)
    w_dec = np.ascontiguousarray(
        attn_w[:DH].reshape(KC, P, D).transpose(1, 0, 2).reshape(P, KC * D)
    )
    # w_enc [E2, D] -> [P, EC*D] with free index (ec, d), pre-cast to bf16
    w_enc = np.ascontiguousarray(
        attn_w[DH:].reshape(EC, P, D).transpose(1, 0, 2).reshape(P, EC * D)
    ).astype(ml_dtypes.bfloat16)

    sel_np = np.zeros((BC, BC * P), dtype=ml_dtypes.bfloat16)
    for b in range(BC):
        sel_np[b, b * P : (b + 1) * P] = 1.0
    bv = np.concatenate([attn_b, v_w]).reshape(1, 2 * D).astype(ml_dtypes.bfloat16)

    kept = [np.nonzero(mask[gb])[0] for gb in range(B)]

    in_maps = []
    for c in range(N_CORES):
        enc_c = np.zeros((P, EC * P * ntot), dtype=ml_dtypes.bfloat16)
        madd = np.full((P, ntot), NEG_BIG, dtype=np.float32)
        perm = assign[c]
        h_t = (
            h[perm]
            .T.reshape(KC, P, BC)
            .transpose(1, 0, 2)
            .reshape(P, KC * BC)
        )
        hwdec = np.concatenate([h_t, w_dec], axis=1).astype(ml_dtypes.bfloat16)
        col = 0
        for b in range(BC):
            gb = perm[b]
            W = widths[b] * P
            idx = kept[gb]
            n = len(idx)
            # kept enc columns, feature-major, padded: [EC, P, W]
            padded = np.zeros((EC, P, W), dtype=ml_dtypes.bfloat16)
            cols = enc[idx, gb, :].T.astype(ml_dtypes.bfloat16)
            padded[:, :, :n] = cols.reshape(EC, P, n)
            # group-major column order: per group sg, ec-major block
            off = 0
            for gsz in _group_sizes(widths[b]):
                blk = padded[:, :, off : off + gsz * P]      # [EC, P, Wg]
                w = EC * gsz * P
                enc_c[:, col : col + w] = blk.transpose(1, 0, 2).reshape(P, w)
                off += gsz * P
                col += w
            # compact additive mask: -shift for real columns, -1e10 for pads
            m = np.full(W, -SHIFT, dtype=np.float32)
            m[n:] = NEG_BIG
            madd[:, toff[b] : toff[b] + widths[b]] = m.reshape(widths[b], P).T
        in_maps.append(
            dict(
                enc_t=enc_c, hwdec=hwdec, madd=madd, w_enc=w_enc,
                sel_in=sel_np, bv=bv,
            )
        )
    return in_maps, kept


def run(inputs, trace=False):
    mask = np.asarray(inputs["mask"], dtype=np.int32)
    counts = mask.sum(axis=1)
    assign, widths = plan_assignment(counts)
    nc = build_nc(widths)
    in_maps, kept = shard_inputs(inputs, assign, widths)
    res = run_bass_kernel_spmd(nc, in_maps, list(range(N_CORES)), trace=trace)
    ntot = sum(widths)
    toff = [sum(widths[:i]) for i in range(len(widths))]
    out_full = np.zeros((B, S), dtype=np.float32)
    for c in range(N_CORES):
        vals = res.results[c]["out"].reshape(P, ntot)
        for b in range(BC):
            gb = assign[c][b]
            idx = kept[gb]
            w = widths[b]
            flat = vals[:, toff[b] : toff[b] + w].T.reshape(w * P)
            out_full[gb, idx] = flat[: len(idx)]
    return out_full, res


def kernel(**inputs) -> np.ndarray:
    out, _ = run(inputs, trace=False)
    return out


# revision 13
# speedup vs baseline: 1.1924x; 1.1924x over previous
"""Bass/Trainium2 kernel for nn_Attention_84688165142614 (additive attention).

Computes, for full inputs (B=32, S=2048, EH=512, DH=512):
    enc    = enc_output.transpose(1, 0, 2)                  # [B, S, 2EH]
    energy = tanh(enc @ w_enc + (h @ w_dec) + attn_b)       # [B, S, DH]
    att    = energy @ v_w                                   # [B, S]
    att    = where(mask == 0, -1e10, att)
    out    = softmax(att, axis=1)

Strategy: data-parallel over batch across 8 NeuronCores (4 batches/core),
plus mask-sparsity compaction. The mask is ~50% zeros and masked positions
produce exactly 0 in the reference output (exp(-1e10) underflows in f32),
so the host keeps only unmasked source positions per batch (gather),
pads each batch to a multiple of 128 columns, transposes the kept enc
columns feature-major and pre-casts to bf16. Batches are assigned to
(core, slot) by sorted compacted width, so the SPMD per-slot tile counts
are the max over cores of the k-th widest batch — for a p=0.5 mask this
drops one padded 128-column tile from most slots. The device computes
energies/logits/softmax only for the compacted columns (pads are killed
with a -1e10 additive mask row), and the host scatters the compacted
probabilities back into a zero [B, S] output.

Device structure: the big matmul runs in bf16 with fp32 PSUM accumulation.
The enc DRAM shard is laid out group-major — exactly the order the PE
consumes it (per slot, per PSUM group of 4 s-tiles, ec-major inside) —
so each (slot, group) is one contiguous DMA into its own SBUF tile, all
on the single sync HWDGE ring in consumption order (SDMA round-robins
across rings at packet granularity, so a single FIFO ring is the only
way to prioritize; it still spreads across all 16 SDMA engines). Warmup
matmuls on memset data keep the PE busy (and the HAM clock un-throttled)
during the initial fill. tanh output and v are bf16. Softmax skips
max-subtraction (logits are bounded by sum|v| ~ 8); all logits are
shifted by -8*ln2 (softmax is shift-invariant) so exp sums fit fp16,
letting the partition-sum broadcast matmul run 1-pass fp16 instead of
2-pass fp32. The last slot folds the dec add into the matmul
accumulation so the vector backlog drains before the kernel tail.
"""

import numpy as np
from contextlib import ExitStack

import concourse.bass as bass
import concourse.tile as tile
from concourse import bacc, mybir
from concourse.bass_utils import run_bass_kernel_spmd

# Problem shape (hardcoded; kernel.py must be self-contained).
B, S, E2, DH = 32, 2048, 1024, 512
N_CORES = 8
BC = B // N_CORES        # batches per core = 4
P = 128                  # SBUF partitions
EC = E2 // P             # enc-feature chunks = 8
D = DH                   # 512
KC = DH // P             # dec-feature chunks = 4

f32 = mybir.dt.float32
bf16 = mybir.dt.bfloat16
fp16 = mybir.dt.float16
AF = mybir.ActivationFunctionType
ALU = mybir.AluOpType

NEG_BIG = -1.0e10
SHIFT = 8.0 * 0.6931471805599453  # logit shift so exp sums fit fp16

_NC_CACHE = {}


def _group_sizes(nt):
    sizes = [4] * (nt // 4)
    if nt % 4:
        sizes.append(nt % 4)
    return sizes


def _emit(ctx, tc, nc, widths, enc_t, hwdec, madd_in, w_enc, sel_in, bv_in, out):
    nslots = len(widths)
    toff = [sum(widths[:i]) for i in range(nslots)]        # tile offset per slot
    ntot = sum(widths)
    slot_sizes = [_group_sizes(w) for w in widths]
    # column offset (in the EC-major enc layout) of (slot, group)
    coff = {}
    c = 0
    for b in range(nslots):
        for sg, gsz in enumerate(slot_sizes[b]):
            coff[(b, sg)] = c
            c += EC * gsz * P

    const = ctx.enter_context(tc.tile_pool(name="const", bufs=1))
    spsum = ctx.enter_context(tc.tile_pool(name="spsum", bufs=1, space="PSUM"))
    mpsum = ctx.enter_context(tc.tile_pool(name="mpsum", bufs=7, space="PSUM"))
    ngrp = sum(len(s) for s in slot_sizes)
    encp = ctx.enter_context(tc.tile_pool(name="encp", bufs=ngrp))
    tmpp = ctx.enter_context(tc.tile_pool(name="tmpp", bufs=3))
    thp = ctx.enter_context(tc.tile_pool(name="thp", bufs=5))
    epip = ctx.enter_context(tc.tile_pool(name="epip", bufs=12))

    # ---- warmup source tiles (no DMA deps): keep the PE busy during fill ----
    ones16 = const.tile([P, P], fp16)
    nc.vector.memset(ones16[:], 1.0)
    wsrc = const.tile([P, D], fp16)
    nc.vector.memset(wsrc[:], 0.001)
    ones_row = const.tile([1, P], bf16)
    nc.vector.memset(ones_row[:], 1.0)

    # ---- DMA: single sync ring in exact consumption order ----
    gtiles = {}
    for b in range(nslots):
        for sg, gsz in enumerate(slot_sizes[b]):
            gtiles[(b, sg)] = encp.tile(
                [P, EC * gsz * P], bf16, tag="enc", name=f"enc_{b}_{sg}"
            )

    wq = const.tile([P, EC * D], bf16)
    hwdec_sb = const.tile([P, KC * BC + KC * D], bf16)
    madd_sb = const.tile([P, ntot], f32)
    sel_sb = const.tile([BC, BC * P], bf16)
    bv_sb = const.tile([1, 2 * D], bf16)

    g00 = gtiles[(0, 0)]
    Wg0 = slot_sizes[0][0] * P
    # fine-grained opening: wq/enc chunk pairs in consumption order
    nc.sync.dma_start(out=wq[:, 0:D], in_=w_enc[:, 0:D])
    nc.sync.dma_start(out=g00[:, 0:Wg0], in_=enc_t[:, 0:Wg0])
    nc.sync.dma_start(out=wq[:, D : 2 * D], in_=w_enc[:, D : 2 * D])
    nc.sync.dma_start(out=g00[:, Wg0 : 2 * Wg0], in_=enc_t[:, Wg0 : 2 * Wg0])
    nc.sync.dma_start(out=wq[:, 2 * D : 4 * D], in_=w_enc[:, 2 * D : 4 * D])
    nc.sync.dma_start(out=g00[:, 2 * Wg0 : 4 * Wg0], in_=enc_t[:, 2 * Wg0 : 4 * Wg0])
    nc.sync.dma_start(out=wq[:, 4 * D : 8 * D], in_=w_enc[:, 4 * D : 8 * D])
    nc.sync.dma_start(out=g00[:, 4 * Wg0 : 8 * Wg0], in_=enc_t[:, 4 * Wg0 : 8 * Wg0])

    # small consts on the scalar ring (kept free of large transfers)
    nc.scalar.dma_start(out=hwdec_sb[:], in_=hwdec[:])
    nc.scalar.dma_start(out=madd_sb[:], in_=madd_in[:])
    nc.scalar.dma_start(out=sel_sb[:], in_=sel_in[:])
    nc.scalar.dma_start(out=bv_sb[:], in_=bv_in[:])

    # remaining groups on the sync ring in consumption order
    rest = [(b, sg) for b in range(nslots) for sg in range(len(slot_sizes[b]))][1:]
    for b, sg in rest:
        gsz = slot_sizes[b][sg]
        lo = coff[(b, sg)]
        hi = lo + EC * gsz * P
        nc.sync.dma_start(out=gtiles[(b, sg)][:], in_=enc_t[:, lo:hi])

    # ---- PE warmup: 8 matmuls on memset data (~3.4us cold, HAM -> 8/8) ----
    wps = spsum.tile([P, D], f32, tag="sp", name="warm")
    for i in range(8):
        nc.tensor.matmul(wps[:], lhsT=ones16[:], rhs=wsrc[:], start=True, stop=True)

    # ---- dec[b, :] = h[slot b] @ w_dec + attn_b; broadcasts ----
    HW0 = KC * BC  # offset of w_dec columns inside hwdec_sb
    dec_ps = spsum.tile([BC, D], f32, tag="sp")
    for kc in range(KC):
        nc.tensor.matmul(
            dec_ps[:],
            lhsT=hwdec_sb[:, kc * BC : (kc + 1) * BC],
            rhs=hwdec_sb[:, HW0 + kc * D : HW0 + (kc + 1) * D],
            start=(kc == 0),
            stop=False,
        )
    nc.tensor.matmul(
        dec_ps[:], lhsT=ones_row[:, 0:BC], rhs=bv_sb[:, 0:D], start=False, stop=True
    )
    dec_rows = const.tile([BC, D], bf16)
    nc.vector.tensor_copy(dec_rows[:], dec_ps[:])

    dec_bc = const.tile([P, BC * D], f32)
    for b in range(nslots):
        ps = spsum.tile([P, D], f32, tag="sp", name=f"decb_{b}")
        nc.tensor.matmul(
            ps[:], lhsT=sel_sb[:, b * P : (b + 1) * P], rhs=dec_rows[:],
            start=True, stop=True,
        )
        nc.vector.tensor_copy(dec_bc[:, b * D : (b + 1) * D], ps[:])
    v_ps = spsum.tile([P, D], f32, tag="sp")
    nc.tensor.matmul(
        v_ps[:], lhsT=ones_row[:], rhs=bv_sb[:, D : 2 * D], start=True, stop=True
    )
    v_sb = const.tile([P, D], bf16)
    nc.vector.tensor_copy(v_sb[:], v_ps[:])

    # ---- main loop over slots ----
    for b in range(nslots):
        nt = widths[b]
        sizes = slot_sizes[b]
        starts = [sum(sizes[:i]) for i in range(len(sizes))]
        # last slot: dec add folded into the accumulation so tanh reads PSUM
        # and the vector backlog drains before the kernel tail
        dec_in_mm = b == nslots - 1
        att = epip.tile([P, nt], f32, tag="att", name=f"att_{b}")
        for sg, gsz in enumerate(sizes):
            gt = gtiles[(b, sg)]
            Wg = gsz * P
            psums = [
                mpsum.tile([P, D], f32, tag="mm", name=f"mm_{b}_{sg}_{j}")
                for j in range(gsz)
            ]
            for ec in range(EC):
                for j in range(gsz):
                    nc.tensor.matmul(
                        psums[j][:],
                        lhsT=gt[:, ec * Wg + j * P : ec * Wg + (j + 1) * P],
                        rhs=wq[:, ec * D : (ec + 1) * D],
                        start=(ec == 0),
                        stop=(ec == EC - 1) and not dec_in_mm,
                    )
            if dec_in_mm:
                for j in range(gsz):
                    nc.tensor.matmul(
                        psums[j][:],
                        lhsT=sel_sb[:, b * P : (b + 1) * P],
                        rhs=dec_rows[:],
                        start=False,
                        stop=True,
                    )
            for j in range(gsz):
                st = starts[sg] + j
                th = thp.tile([P, D], bf16, tag="th")
                if dec_in_mm:
                    nc.scalar.activation(th[:], psums[j][:], AF.Tanh)
                else:
                    t_sb = tmpp.tile([P, D], f32, tag="tmp")
                    nc.vector.tensor_add(
                        t_sb[:], psums[j][:], dec_bc[:, b * D : (b + 1) * D]
                    )
                    nc.scalar.activation(th[:], t_sb[:], AF.Tanh)
                scr = thp.tile([P, D], bf16, tag="scr")
                nc.vector.affine_mul_reduce(
                    out=scr[:],
                    accum_out=att[:, st : st + 1],
                    in0=th[:],
                    in1=v_sb[:],
                    scale=1.0,
                    bias=0.0,
                )

        # ---- epilogue: mask+shift, exp, fp16 partition-sum bcast, scale ----
        attm = epip.tile([P, nt], f32, tag="attm", name=f"attm_{b}")
        nc.vector.tensor_add(
            attm[:], att[:], madd_sb[:, toff[b] : toff[b] + nt]
        )
        expt = epip.tile([P, nt], f32, tag="expt", name=f"expt_{b}")
        nc.scalar.activation(expt[:], attm[:], AF.Exp)
        p16 = epip.tile([P, 1], fp16, tag="p16", name=f"p16_{b}")
        with nc.allow_low_precision(reason="fp16 softmax denominator rounding"):
            nc.vector.tensor_reduce(
                p16[:], expt[:], mybir.AxisListType.X, ALU.add
            )
        tot_ps = spsum.tile([P, 1], f32, tag="sp", name=f"tot_{b}")
        nc.tensor.matmul(
            tot_ps[:], lhsT=ones16[:], rhs=p16[:], start=True, stop=True
        )
        r_pp = epip.tile([P, 1], f32, tag="rpp", name=f"rpp_{b}")
        nc.vector.reciprocal(r_pp[:], tot_ps[:])
        out_sb = epip.tile([P, nt], f32, tag="outsb", name=f"osb_{b}")
        nc.vector.tensor_scalar_mul(out_sb[:], expt[:], r_pp[:])
        nc.scalar.dma_start(
            out=out[:, toff[b] : toff[b] + nt], in_=out_sb[:]
        )


def build_nc(widths):
    key = tuple(widths)
    if key in _NC_CACHE:
        return _NC_CACHE[key]
    ntot = sum(widths)
    nc = bacc.Bacc("TRN2", target_bir_lowering=False, debug=False)
    enc_t = nc.dram_tensor(
        "enc_t", [P, EC * P * ntot], bf16, kind="ExternalInput"
    ).ap()
    hwdec = nc.dram_tensor(
        "hwdec", [P, KC * BC + KC * D], bf16, kind="ExternalInput"
    ).ap()
    madd = nc.dram_tensor("madd", [P, ntot], f32, kind="ExternalInput").ap()
    w_enc = nc.dram_tensor("w_enc", [P, EC * D], bf16, kind="ExternalInput").ap()
    sel_in = nc.dram_tensor("sel_in", [BC, BC * P], bf16, kind="ExternalInput").ap()
    bv = nc.dram_tensor("bv", [1, 2 * D], bf16, kind="ExternalInput").ap()
    out = nc.dram_tensor("out", [P, ntot], f32, kind="ExternalOutput").ap()

    with tile.TileContext(nc) as tc:
        with ExitStack() as ctx:
            _emit(ctx, tc, nc, list(widths), enc_t, hwdec, madd, w_enc, sel_in,
                  bv, out)
    nc.compile()
    _NC_CACHE[key] = nc
    return nc


def plan_assignment(counts):
    """Sort batches by compacted tile count; rank k -> core k%8, slot k//8.
    Returns (assign[core][slot] = global batch, widths[slot])."""
    tiles = np.maximum(1, np.ceil(counts / P).astype(int))
    order = sorted(range(B), key=lambda gb: (-tiles[gb], -counts[gb], gb))
    assign = [[-1] * BC for _ in range(N_CORES)]
    widths = []
    for slot in range(BC):
        ranks = order[slot * N_CORES : (slot + 1) * N_CORES]
        for c, gb in enumerate(ranks):
            assign[c][slot] = gb
        widths.append(max(int(tiles[gb]) for gb in ranks))
    return assign, widths


def shard_inputs(inputs, assign, widths):
    import ml_dtypes

    h = np.asarray(inputs["h"], dtype=np.float32)
    enc = np.asarray(inputs["enc_output"], dtype=np.float32)
    mask = np.asarray(inputs["mask"], dtype=np.int32)
    attn_w = np.asarray(inputs["attn_w"], dtype=np.float32)
    attn_b = np.asarray(inputs["attn_b"], dtype=np.float32)
    v_w = np.asarray(inputs["v_w"], dtype=np.float32)

    ntot = sum(widths)
    toff = [sum(widths[:i]) for i in range(len(widths))]

    # w_dec [DH, D] -> [P, KC*D] with free index (kc, d)
    w_dec = np.ascontiguousarray(
        attn_w[:DH].reshape(KC, P, D).transpose(1, 0, 2).reshape(P, KC * D)
    )
    # w_enc [E2, D] -> [P, EC*D] with free index (ec, d), pre-cast to bf16
    w_enc = np.ascontiguousarray(
        attn_w[DH:].reshape(EC, P, D).transpose(1, 0, 2).reshape(P, EC * D)
    ).astype(ml_dtypes.bfloat16)

    sel_np = np.zeros((BC, BC * P), dtype=ml_dtypes.bfloat16)
    for b in range(BC):
        sel_np[b, b * P : (b + 1) * P] = 1.0
    bv = np.concatenate([attn_b, v_w]).reshape(1, 2 * D).astype(ml_dtypes.bfloat16)

    kept = [np.nonzero(mask[gb])[0] for gb in range(B)]

    in_maps = []
    for c in range(N_CORES):
        enc_c = np.zeros((P, EC * P * ntot), dtype=ml_dtypes.bfloat16)
        madd = np.full((P, ntot), NEG_BIG, dtype=np.float32)
        perm = assign[c]
        h_t = (
            h[perm]
            .T.reshape(KC, P, BC)
            .transpose(1, 0, 2)
            .reshape(P, KC * BC)
        )
        hwdec = np.concatenate([h_t, w_dec], axis=1).astype(ml_dtypes.bfloat16)
        col = 0
        for b in range(BC):
            gb = perm[b]
            W = widths[b] * P
            idx = kept[gb]
            n = len(idx)
            # kept enc columns, feature-major, padded: [EC, P, W]
            padded = np.zeros((EC, P, W), dtype=ml_dtypes.bfloat16)
            cols = enc[idx, gb, :].T.astype(ml_dtypes.bfloat16)
            padded[:, :, :n] = cols.reshape(EC, P, n)
            # group-major column order: per group sg, ec-major block
            off = 0
            for gsz in _group_sizes(widths[b]):
                blk = padded[:, :, off : off + gsz * P]      # [EC, P, Wg]
                w = EC * gsz * P
                enc_c[:, col : col + w] = blk.transpose(1, 0, 2).reshape(P, w)
                off += gsz * P
                col += w
            # compact additive mask: -shift for real columns, -1e10 for pads
            m = np.full(W, -SHIFT, dtype=np.float32)
            m[n:] = NEG_BIG
            madd[:, toff[b] : toff[b] + widths[b]] = m.reshape(widths[b], P).T
        in_maps.append(
            dict(
                enc_t=enc_c, hwdec=hwdec, madd=madd, w_enc=w_enc,
                sel_in=sel_np, bv=bv,
            )
        )
    return in_maps, kept


def run(inputs, trace=False):
    mask = np.asarray(inputs["mask"], dtype=np.int32)
    counts = mask.sum(axis=1)
    assign, widths = plan_assignment(counts)
    nc = build_nc(widths)
    in_maps, kept = shard_inputs(inputs, assign, widths)
    res = run_bass_kernel_spmd(nc, in_maps, list(range(N_CORES)), trace=trace)
    ntot = sum(widths)
    toff = [sum(widths[:i]) for i in range(len(widths))]
    out_full = np.zeros((B, S), dtype=np.float32)
    for c in range(N_CORES):
        vals = res.results[c]["out"].reshape(P, ntot)
        for b in range(BC):
            gb = assign[c][b]
            idx = kept[gb]
            w = widths[b]
            flat = vals[:, toff[b] : toff[b] + w].T.reshape(w * P)
            out_full[gb, idx] = flat[: len(idx)]
    return out_full, res


def kernel(**inputs) -> np.ndarray:
    out, _ = run(inputs, trace=False)
    return out
